# revision 1
# baseline (speedup 1.0000x reference)
"""CTC loss (Keras ctc_batch_cost semantics) on 8 Trainium2 NeuronCores.

Strategy
--------
Data-parallel over batch: each core takes 32 of the 256 sequences.

Per core, the CTC forward DP runs in log space with states laid out on
SBUF *partitions* (s = extended-label position) and (example, direction)
on the free dim.  A forward chain (t = 0..255) and a backward chain
(t = 511..256, states s-reversed so the shifts point the same way) are
stacked into one 64-column state, so every DVE instruction advances both
chains for all 32 examples at once.  The state shifts along s are
constant shift-matrix matmuls on the (otherwise idle) TensorEngine.

Deferred-log representation: alpha = W + log(S) with S in [1, 3^64].
One DP step (pair j) computes the 3-way log-sum-exp
    A'[s] = lp[t, s] + log( e^{A[s]} + e^{A[s-1]} + gate(s) e^{A[s-2]} )
as: W-part maxes/subs on DVE, ONE batched ACT Exp over [128, 3*64]
(always warm - no table switching), three products + two adds for S',
and W' = max-center + lp.  log(S) is only materialised every 64 pairs
(range flush) and on the host at the end - the per-step ACT Ln that
would otherwise thrash the activation tables is gone.  No
renormalisation is needed in log space.

The gather lp[t,s] = log(y_pred[t, ext[s]] + eps) is produced on-device:
PE transposes y_pred chunks ([t,v] -> [v,t], with an anti-diagonal
"identity" for the backward direction, which time-reverses for free),
then a one-hot matmul per (example, direction) gathers the needed
columns (+eps folded into the one-hot matrix: G = onehot + eps, exact
because softmax rows sum to 1), and an ACT Ln writes the lp tile.

The forward chain covers states s=0..127 (dropping s=128, which never
feeds other states forward) and the backward chain covers s=1..128
(dropping s=0).  Host combine in float64:
loss[b] = -logsumexp_{s=1..127}(alpha_255[s] + betahat_255[s])
(endpoint terms negligible; validated to max rel err ~1e-6 vs the
f32 reference).
"""

import sys

sys.path.insert(0, "/opt/trn_rl_repo")

from contextlib import ExitStack

import numpy as np

import concourse.bass as bass
import concourse.tile as tile
from concourse import bacc, mybir
from concourse.bass_utils import run_bass_kernel_spmd

B, T, V, L = 256, 512, 256, 64
S = 2 * L + 1        # 129 extended states
BLANK = V - 1
EPS = 1e-7
NEGF = -1.0e30
NCORES = 8
BPC = B // NCORES    # 32 examples per core
NPAIRS = T // 2      # 256 step-pairs (j=0 init, j=1..255 step, +1 extra)
KFLUSH = 40          # S-range flush period (S <= 3^40 = 1.2e19 < 2^64 ACT Ln range)
FP32 = mybir.dt.float32
AF = mybir.ActivationFunctionType
ALU = mybir.AluOpType


def _kernel_body(ctx, tc, y_in, g_in, supd2_in, supd1_in, cst_in,
                 outaw, outas, outbw, outbs):
    nc = tc.nc

    const_pool = ctx.enter_context(tc.tile_pool(name="const", bufs=1))
    g_pool = ctx.enter_context(tc.tile_pool(name="gmat", bufs=1))
    lp_pool = ctx.enter_context(tc.tile_pool(name="lp", bufs=1))
    ystage = ctx.enter_context(tc.tile_pool(name="ystage", bufs=3))
    yt_pool = ctx.enter_context(tc.tile_pool(name="yt", bufs=3))
    psum_tr = ctx.enter_context(tc.tile_pool(name="ptr", bufs=2, space="PSUM"))
    psum_g = ctx.enter_context(tc.tile_pool(name="pg", bufs=2, space="PSUM"))
    psum_w = ctx.enter_context(tc.tile_pool(name="pshw", bufs=1, space="PSUM"))
    psum_s = ctx.enter_context(tc.tile_pool(name="pshs", bufs=1, space="PSUM"))
    state = ctx.enter_context(tc.tile_pool(name="state", bufs=3))
    work = ctx.enter_context(tc.tile_pool(name="work", bufs=3))

    # --- constants: cst_in = [SH1, SH2, I, J] ---
    cst = const_pool.tile([128, 4, 128], FP32)
    nc.sync.dma_start(cst[:], cst_in.rearrange("k p f -> p k f"))
    sh1 = cst[:, 0, :]
    sh2 = cst[:, 1, :]
    idn = cst[:, 2, :]
    jdn = cst[:, 3, :]
    supd1 = const_pool.tile([128, 1], FP32)
    nc.sync.dma_start(supd1[:], supd1_in[:])
    supd2 = const_pool.tile([128, 64], FP32)
    nc.sync.dma_start(supd2[:], supd2_in[:])

    # --- one-hot gather matrices, resident ---
    gm = g_pool.tile([128, 2, BPC, 2, 128], FP32)
    nc.sync.dma_start(gm[:], g_in.rearrange("d b h v s -> v d b h s"))

    # --- lp tiles: [s=128 part | pair j, (b,dir) col] ---
    lp = lp_pool.tile([128, NPAIRS, 64], FP32)

    def produce_chunk(j0, jn):
        # Demote producer priority so the serial DP chain wins scheduler
        # ties on the shared ACT/PE engines; producer fills real gaps only.
        _save_pri = tc.cur_priority
        tc.cur_priority = _save_pri + 1_000_000
        # anti-diagonal / identity slices sized to the t-block
        idn_s = idn[0:jn, 0:jn]
        jdn_s = jdn[0:jn, 128 - jn:128]
        for b_ in range(BPC):
            for d_ in range(2):             # 0 = fwd, 1 = bwd
                t0 = j0 if d_ == 0 else T - j0 - jn
                ystg = ystage.tile([jn, 256], FP32, tag="ystg")
                nc.sync.dma_start(ystg[:], y_in[b_, t0:t0 + jn, :])
                gps = psum_g.tile([128, jn], FP32, tag="gps")
                for h in range(2):          # v halves
                    ptr = psum_tr.tile([128, jn], FP32, tag="ptr")
                    nc.tensor.transpose(
                        ptr[:], ystg[:, 128 * h:128 * (h + 1)],
                        idn_s if d_ == 0 else jdn_s)
                    ytt = yt_pool.tile([128, jn], FP32, tag="ytt")
                    nc.scalar.copy(ytt[:], ptr[:])
                    nc.tensor.matmul(
                        gps[:], gm[:, d_, b_, h, :], ytt[:],
                        start=(h == 0), stop=(h == 1))
                col = b_ + BPC * d_
                nc.scalar.activation(
                    lp[:, j0:j0 + jn, col], gps[:], AF.Ln)
        tc.cur_priority = _save_pri

    produce_chunk(0, 128)

    # --- DP chain: state (W, S), alpha = W + log S ---
    w_t = state.tile([128, 64], FP32, tag="w")
    nc.vector.memset(w_t[:], NEGF)
    nc.vector.tensor_copy(w_t[0:2, :], lp[0:2, 0, :])
    s_t = state.tile([128, 64], FP32, tag="s")
    nc.vector.memset(s_t[:], 1.0)
    for j in range(1, NPAIRS + 1):
        if j == 24:
            produce_chunk(128, 128)
        extra = (j == NPAIRS)
        p1w = psum_w.tile([128, 64], FP32, tag="p1w")
        nc.tensor.matmul(p1w[:], sh1, w_t[:], start=True, stop=True)
        p1s = psum_s.tile([128, 64], FP32, tag="p1s")
        nc.tensor.matmul(p1s[:], sh1, s_t[:], start=True, stop=True)
        p2w = psum_w.tile([128, 64], FP32, tag="p2w")
        nc.tensor.matmul(p2w[:], sh2, w_t[:], start=True, stop=True)
        p2s = psum_s.tile([128, 64], FP32, tag="p2s")
        nc.tensor.matmul(p2s[:], sh2, s_t[:], start=True, stop=True)

        g2 = work.tile([128, 64], FP32, tag="g2")
        nc.vector.tensor_add(g2[:], p2w[:], supd2[:])
        mx12 = work.tile([128, 64], FP32, tag="mx12")
        nc.vector.scalar_tensor_tensor(
            mx12[:], p1w[:], supd1[:], w_t[:], ALU.add, ALU.max)
        mx3 = work.tile([128, 64], FP32, tag="mx3")
        nc.vector.tensor_max(mx3[:], mx12[:], g2[:])

        dd = work.tile([128, 192], FP32, tag="dd")
        nc.vector.tensor_sub(dd[:, 0:64], w_t[:], mx3[:])
        nc.vector.scalar_tensor_tensor(
            dd[:, 64:128], p1w[:], supd1[:], mx3[:], ALU.add, ALU.subtract)
        nc.vector.tensor_sub(dd[:, 128:192], g2[:], mx3[:])
        ee = work.tile([128, 192], FP32, tag="ee")
        nc.scalar.activation(ee[:], dd[:], AF.Exp)

        t0_ = work.tile([128, 64], FP32, tag="t0")
        nc.vector.tensor_mul(t0_[:], ee[:, 0:64], s_t[:])
        t1_ = work.tile([128, 64], FP32, tag="t1")
        nc.vector.tensor_mul(t1_[:], ee[:, 64:128], p1s[:])
        t2_ = work.tile([128, 64], FP32, tag="t2")
        nc.vector.tensor_mul(t2_[:], ee[:, 128:192], p2s[:])
        u_ = work.tile([128, 64], FP32, tag="u")
        nc.vector.tensor_add(u_[:], t0_[:], t1_[:])
        s_new = state.tile([128, 64], FP32, tag="s")
        nc.vector.tensor_add(s_new[:], u_[:], t2_[:])
        w_new = state.tile([128, 64], FP32, tag="w")
        if extra:
            nc.vector.tensor_copy(w_new[:], mx3[:])
        else:
            nc.vector.tensor_add(w_new[:], mx3[:], lp[:, j, :])

        if j % KFLUSH == 0 and not extra:
            ls_ = work.tile([128, 64], FP32, tag="ls")
            nc.scalar.activation(ls_[:], s_new[:], AF.Ln)
            w2 = state.tile([128, 64], FP32, tag="w")
            nc.vector.tensor_add(w2[:], w_new[:], ls_[:])
            s2 = state.tile([128, 64], FP32, tag="s")
            nc.vector.memset(s2[:], 1.0)
            w_new, s_new = w2, s2

        if j == NPAIRS - 1:
            nc.sync.dma_start(outaw[:], w_new[:])
            nc.sync.dma_start(outas[:], s_new[:])
        if extra:
            nc.sync.dma_start(outbw[:], w_new[:])
            nc.sync.dma_start(outbs[:], s_new[:])
        w_t, s_t = w_new, s_new


_CACHED = None


def _build():
    global _CACHED
    if _CACHED is not None:
        return _CACHED
    nc = bacc.Bacc("TRN2", target_bir_lowering=False, debug=False,
                   num_devices=NCORES)
    y_in = nc.dram_tensor("y", [BPC, T, V], FP32, kind="ExternalInput").ap()
    g_in = nc.dram_tensor("g", [2, BPC, 2, 128, 128], FP32,
                          kind="ExternalInput").ap()
    supd2_in = nc.dram_tensor("supd2", [128, 64], FP32,
                              kind="ExternalInput").ap()
    supd1_in = nc.dram_tensor("supd1", [128, 1], FP32,
                              kind="ExternalInput").ap()
    cst_in = nc.dram_tensor("cst", [4, 128, 128], FP32,
                            kind="ExternalInput").ap()
    outaw = nc.dram_tensor("outaw", [128, 64], FP32, kind="ExternalOutput").ap()
    outas = nc.dram_tensor("outas", [128, 64], FP32, kind="ExternalOutput").ap()
    outbw = nc.dram_tensor("outbw", [128, 64], FP32, kind="ExternalOutput").ap()
    outbs = nc.dram_tensor("outbs", [128, 64], FP32, kind="ExternalOutput").ap()

    with tile.TileContext(nc) as tc:
        with ExitStack() as ctx:
            _kernel_body(ctx, tc, y_in, g_in, supd2_in, supd1_in, cst_in,
                         outaw, outas, outbw, outbs)
    nc.compile()
    _CACHED = nc
    return nc


def _host_tensors(y_true, y_pred):
    """Per-core input dicts (everything derived from y_true is host-side
    index preprocessing; all FLOP-carrying work runs on device)."""
    y_true = np.asarray(y_true)
    y_pred = np.ascontiguousarray(np.asarray(y_pred, dtype=np.float32))

    sh1 = np.zeros((128, 128), np.float32)
    sh1[np.arange(127), np.arange(1, 128)] = 1.0
    sh2 = np.zeros((128, 128), np.float32)
    sh2[np.arange(126), np.arange(2, 128)] = 1.0
    idn = np.eye(128, dtype=np.float32)
    jdn = np.fliplr(np.eye(128)).astype(np.float32)
    cst = np.stack([sh1, sh2, idn, jdn]).astype(np.float32)

    supd1 = np.zeros((128, 1), np.float32)
    supd1[0, 0] = NEGF

    in_maps = []
    for core in range(NCORES):
        bs = slice(core * BPC, (core + 1) * BPC)
        yt_c = y_true[bs]
        g = np.full((2, BPC, 2, 128, 128), EPS, dtype=np.float32)
        supd2 = np.full((128, 64), NEGF, dtype=np.float32)
        for b_ in range(BPC):
            ext = np.full(S, BLANK, dtype=np.int64)
            ext[1::2] = yt_c[b_]
            extm2 = np.concatenate([np.full(2, -1, dtype=np.int64), ext[:-2]])
            skip = (ext != BLANK) & (ext != extm2)          # [S]
            # fwd: columns s = 0..127
            for s_ in range(128):
                v = ext[s_]
                g[0, b_, v // 128, v % 128, s_] += 1.0
            # bwd: columns r = 0..127 <-> s = 128 - r
            for r_ in range(128):
                v = ext[128 - r_]
                g[1, b_, v // 128, v % 128, r_] += 1.0
            # destination gates
            sarr = np.arange(2, 128)
            supd2[sarr[skip[2:128]], b_] = 0.0
            rarr = np.arange(2, 128)
            src_s = 130 - rarr                              # in [3, 128]
            supd2[rarr[skip[src_s]], BPC + b_] = 0.0
        in_maps.append({
            "y": np.ascontiguousarray(y_pred[bs]),
            "g": g,
            "supd2": supd2,
            "supd1": supd1,
            "cst": cst,
        })
    return in_maps


def _combine(aw, as_, bw, bs_):
    """Host f64 combine: loss[b] = -logsumexp_s(alpha[s] + betahat[s])."""
    loss = np.zeros(B, dtype=np.float64)
    for core in range(NCORES):
        a64 = aw[core].astype(np.float64) + np.log(as_[core].astype(np.float64))
        b64 = bw[core].astype(np.float64) + np.log(bs_[core].astype(np.float64))
        for b_ in range(BPC):
            al = a64[:, b_]                 # alpha_255[s], s = 0..127
            bt = b64[:, BPC + b_]           # betahat[r],   s = 128 - r
            ls = al[1:128] + bt[127:0:-1]   # s = 1..127
            mm = ls.max()
            loss[core * BPC + b_] = -(np.log(np.exp(ls - mm).sum()) + mm)
    return loss


def kernel(y_true, y_pred):
    nc = _build()
    in_maps = _host_tensors(y_true, y_pred)
    res = run_bass_kernel_spmd(nc, in_maps, list(range(NCORES)))
    aw = [res.results[i]["outaw"] for i in range(NCORES)]
    as_ = [res.results[i]["outas"] for i in range(NCORES)]
    bw = [res.results[i]["outbw"] for i in range(NCORES)]
    bs_ = [res.results[i]["outbs"] for i in range(NCORES)]
    loss = _combine(aw, as_, bw, bs_)
    return loss.astype(np.float32)[:, None]



# revision 5
# speedup vs baseline: 2.0879x; 2.0879x over previous
"""CTC loss (Keras ctc_batch_cost semantics) on 8 Trainium2 NeuronCores.

Strategy (v2)
-------------
Data-parallel over batch: each core takes 32 of the 256 sequences, and
runs the fwd chain (t=0..255) and the bwd chain (t=511..256, states
reversed) together as 64 rows of one transposed-layout DP.

The DP runs in PROBABILITY space: with (b,dir) on SBUF partitions and
the extended-label state s on the free dimension, one time step is

    S'[r, s] = q_j[r,s] * (S[r,s] + S[r,s-1]) + qg_j[r,s] * S[r,s-2]

where q = y_pred[., t, ext[s]] + eps (gathered emission probs) and
qg = q * skip-gate.  The state shifts are free-dim AP offsets (an
overlapping stride -1 view), so a step is 4 bf16 DVE instructions and
nothing else -- no matmuls, no PSUM, no log/exp in the serial chain.

fp32/bf16 range is handled by a flush every KF steps: the max of S
over a window around the wavefront diagonal (s ~ j/2) is rescaled to
2^BIAS by an exact power of two (exponent bit arithmetic on DVE), a
high cap protects runaway leader states, and the applied log2-scale
accumulates per row.  States that underflow relative to the window are
> e^-45 below every contributing path -- dropping them is harmless at
the 2e-2 tolerance (validated vs the f32 reference at ~6e-3 max rel
err over all 256 examples in numpy simulation of this exact bf16
arithmetic).

The q/qg tables are produced on device, overlapped with the DP:
PE transposes y chunks ([t,v] -> [v,t], fp32), an ACT/Pool copy
converts to bf16, one-hot gather matmuls (bf16, eps folded in, gate
folded into the qg columns) produce [t, 2, 128] tiles, and a single
DMA per (b,dir,chunk) scatters them into the per-row [j, 2, 128] qcat
layout (time-reversed for the bwd rows via a negative j stride).

Host combine in f64: loss = -logsumexp_s(alpha[s] + betahat[s]),
alpha = log(S_fwd) - E_fwd*log 2, exactly as the validated v1 combine.
"""

import sys

sys.path.insert(0, "/opt/trn_rl_repo")

from contextlib import ExitStack

import numpy as np
import ml_dtypes

import concourse.bass as bass
import concourse.tile as tile
from concourse import bacc, mybir
from concourse.ap import AP
from concourse.bass_utils import run_bass_kernel_spmd

bf16 = ml_dtypes.bfloat16

B, T, V, L = 256, 512, 256, 64
S = 2 * L + 1            # 129 extended states; chains keep 128 each
BLANK = V - 1
EPS = 1e-7
NCORES = 8
BPC = B // NCORES        # 32 examples per core
NJ = T // 2              # 256 time steps per chain
KF = 8                   # flush period
BIAS = 64                # flush rescales window max to 2^BIAS
WIN = 24                 # flush window half-width around the diagonal
CAPF = float(2.0 ** 101)
FP32 = mybir.dt.float32
BF16 = mybir.dt.bfloat16
I32 = mybir.dt.int32
ALU = mybir.AluOpType


def _kernel_body(ctx, tc, y_in, g_in, cst_in, s_out, e_out):
    nc = tc.nc

    const_pool = ctx.enter_context(tc.tile_pool(name="const", bufs=1))
    g_pool = ctx.enter_context(tc.tile_pool(name="gmat", bufs=1))
    qcat_pool = ctx.enter_context(tc.tile_pool(name="qcat", bufs=1))
    ystage = ctx.enter_context(tc.tile_pool(name="ystage", bufs=3))
    yt_pool = ctx.enter_context(tc.tile_pool(name="yt", bufs=3))
    qs_pool = ctx.enter_context(tc.tile_pool(name="qs", bufs=3))
    psum_tr = ctx.enter_context(tc.tile_pool(name="ptr", bufs=2, space="PSUM"))
    psum_g = ctx.enter_context(tc.tile_pool(name="pg", bufs=2, space="PSUM"))
    work = ctx.enter_context(tc.tile_pool(name="work", bufs=4))

    # transpose matrices: identity (fwd) and anti-diagonal (bwd time-rev)
    cst = const_pool.tile([128, 2, 128], FP32)
    nc.sync.dma_start(cst[:], cst_in.rearrange("k p f -> p k f"))

    # gather matrices, resident: [v, d, b, h, 2*128]
    gm = g_pool.tile([128, 2, BPC, 2, 256], BF16)
    nc.sync.dma_start(gm[:], g_in.rearrange("d b h v c -> v d b h c"))

    # q tables: per row r=(d,b), per step j: [2, 128] (q | q*gate)
    qcat = qcat_pool.tile([64, NJ, 2, 128], BF16)

    # ---- production --------------------------------------------------
    # group ch covers j in [ch*128, (ch+1)*128): fwd rows consume y time
    # chunk ch, bwd rows consume chunk 3-ch (j-reversed within chunk).
    def produce_group(ch):
        _save = tc.cur_priority
        tc.cur_priority = _save + 1_000_000
        chunks = (0, 3) if ch == 0 else (1, 2)
        for b_ in range(BPC):
            # one DMA stages both needed chunks: [t(128), pair(2), v]
            ys = ystage.tile([128, 2, V], FP32, tag="ys")
            ya = y_in[b_]
            src = AP(ya.tensor, ya.offset + chunks[0] * 128 * V,
                     [[V, 128], [(chunks[1] - chunks[0]) * 128 * V, 2], [1, V]])
            nc.sync.dma_start(ys[:], src)
            for d_ in range(2):
                ysl = ys[:, d_, :]              # [128, 256] fp32, chunk for d_
                ptr = psum_tr.tile([128, 256], FP32, tag="ptr")
                for h in range(2):
                    nc.tensor.transpose(ptr[:, 128 * h:128 * (h + 1)],
                                        ysl[:, 128 * h:128 * (h + 1)],
                                        cst[:, d_, :])
                yt_ = yt_pool.tile([128, 256], BF16, tag="yt")
                nc.scalar.copy(yt_[:], ptr[:])
                gps = psum_g.tile([128, 256], FP32, tag="gps")
                for h in range(2):
                    nc.tensor.matmul(gps[:], yt_[:, 128 * h:128 * (h + 1)],
                                     gm[:, d_, b_, h, :],
                                     start=(h == 0), stop=(h == 1))
                qs = qs_pool.tile([128, 2, 128], BF16, tag="qs")
                nc.scalar.copy(qs[:], gps[:])
                r_ = d_ * BPC + b_
                a = qcat[r_:r_ + 1, ch * 128:(ch + 1) * 128, :, :]
                dst = AP(a.tensor, a.offset,
                         [list(a.ap[0]), [256, 128], [1, 256]])
                nc.sync.dma_start(dst, qs[:])
        tc.cur_priority = _save

    produce_group(0)

    # ---- DP chain ----------------------------------------------------
    SA = const_pool.tile([64, 130], BF16)
    SB = const_pool.tile([64, 130], BF16)
    eacc = const_pool.tile([64, 1], FP32)
    nc.vector.memset(SA[:], 0.0)
    nc.vector.memset(SB[:], 0.0)
    nc.vector.memset(eacc[:], 0.0)
    nc.vector.tensor_copy(SA[:, 2:4], qcat[:, 0, 0, 0:2])

    cur, nxt = SA, SB
    for j in range(1, NJ):
        if j == 64:
            produce_group(1)
        qj = qcat[:, j]
        q0b = qj[:, 0, :].unsqueeze(1).broadcast_to([64, 2, 128])
        sap = cur[:]
        s2v = AP(sap.tensor, sap.offset + 2,
                 [list(sap.ap[0]), [-1, 2], [1, 128]])
        m = work.tile([64, 3, 128], BF16, tag="m")
        nc.vector.tensor_mul(m[:, 0:2, :], q0b, s2v)
        nc.vector.tensor_mul(m[:, 2, :], qj[:, 1, :], cur[:, 0:128])
        u = work.tile([64, 128], BF16, tag="u")
        nc.vector.tensor_add(u[:], m[:, 0, :], m[:, 1, :])
        nc.vector.tensor_add(nxt[:, 2:130], u[:], m[:, 2, :])
        cur, nxt = nxt, cur

        if j % KF == 0 and j < NJ - 1:
            s0 = j // 2
            lo, hi = max(0, s0 - WIN), min(128, s0 + WIN + 1)
            wm32 = work.tile([64, 1], FP32, tag="wm32")
            nc.vector.tensor_reduce(wm32[:], cur[:, 2 + lo:2 + hi],
                                    axis=mybir.AxisListType.X, op=ALU.max)
            t1 = work.tile([64, 1], I32, tag="t1")
            nc.vector.tensor_scalar(t1[:], wm32[:].bitcast(I32), 23, -1,
                                    op0=ALU.logical_shift_right,
                                    op1=ALU.bitwise_xor)
            f = work.tile([64, 1], I32, tag="f")
            nc.vector.tensor_scalar(f[:], t1[:], BIAS + 255, 254,
                                    op0=ALU.add, op1=ALU.min)
            nc.vector.scalar_tensor_tensor(eacc[:], f[:], -127.0, eacc[:],
                                           ALU.add, ALU.add)
            sc_i = work.tile([64, 1], I32, tag="sci")
            nc.vector.tensor_scalar(sc_i[:], f[:], 23, None,
                                    op0=ALU.logical_shift_left)
            nc.vector.tensor_scalar(nxt[:], cur[:], sc_i[:].bitcast(FP32), CAPF,
                                    op0=ALU.mult, op1=ALU.min)
            cur, nxt = nxt, cur

    nc.sync.dma_start(s_out, cur[:])
    nc.sync.dma_start(e_out, eacc[:])


_CACHED = None


def _build():
    global _CACHED
    if _CACHED is not None:
        return _CACHED
    nc = bacc.Bacc("TRN2", target_bir_lowering=False, debug=False,
                   num_devices=NCORES)
    y_in = nc.dram_tensor("y", [BPC, T, V], FP32, kind="ExternalInput").ap()
    g_in = nc.dram_tensor("g", [2, BPC, 2, 128, 256], BF16,
                          kind="ExternalInput").ap()
    cst_in = nc.dram_tensor("cst", [2, 128, 128], FP32,
                            kind="ExternalInput").ap()
    s_out = nc.dram_tensor("souts", [64, 130], BF16, kind="ExternalOutput").ap()
    e_out = nc.dram_tensor("eouts", [64, 1], FP32, kind="ExternalOutput").ap()

    with tile.TileContext(nc) as tc:
        with ExitStack() as ctx:
            _kernel_body(ctx, tc, y_in, g_in, cst_in, s_out, e_out)
    nc.compile()
    _CACHED = nc
    return nc


def _host_tensors(y_true, y_pred):
    """Per-core input dicts. Only index preprocessing happens on host."""
    y_true = np.asarray(y_true)
    y_pred = np.ascontiguousarray(np.asarray(y_pred, dtype=np.float32))

    idn = np.stack([np.eye(128, dtype=np.float32),
                    np.fliplr(np.eye(128)).astype(np.float32)])

    in_maps = []
    for core in range(NCORES):
        bs = slice(core * BPC, (core + 1) * BPC)
        yt_c = y_true[bs]
        g = np.zeros((2, BPC, 2, 128, 256), np.float32)
        for b_ in range(BPC):
            ext = np.full(S, BLANK, dtype=np.int64)
            ext[1::2] = yt_c[b_]
            extm2 = np.concatenate([np.full(2, -1, dtype=np.int64), ext[:-2]])
            skip = ((ext != BLANK) & (ext != extm2)).astype(np.float32)
            # fwd (d=0): col s = 0..127 from ext[s]
            gf = np.zeros(128, np.float32)
            gf[2:] = skip[2:128]
            vf = ext[0:128]
            # bwd (d=1): col r = 0..127 from ext[128-r]
            gb = np.zeros(128, np.float32)
            rarr = np.arange(2, 128)
            gb[rarr] = skip[130 - rarr]
            vb = ext[128 - np.arange(128)]
            for d_, vv, gg in ((0, vf, gf), (1, vb, gb)):
                for s_ in range(128):
                    v = int(vv[s_])
                    # q column: onehot + eps on every v row
                    g[d_, b_, v // 128, v % 128, s_] += 1.0
                    g[d_, b_, :, :, s_] += EPS
                    # qg column: (onehot + eps) * gate
                    if gg[s_] > 0:
                        g[d_, b_, v // 128, v % 128, 128 + s_] += 1.0
                        g[d_, b_, :, :, 128 + s_] += EPS
        in_maps.append({
            "y": np.ascontiguousarray(y_pred[bs]),
            "g": g.astype(bf16),
            "cst": idn,
        })
    return in_maps


def _combine(souts, eouts):
    """Host f64 combine: loss[b] = -logsumexp_s(alpha[s] + betahat[s])."""
    ln2 = np.log(2.0)
    loss = np.zeros(B, dtype=np.float64)
    with np.errstate(divide="ignore"):
        for core in range(NCORES):
            sv = souts[core].astype(np.float64)
            ev = eouts[core].astype(np.float64)
            for b_ in range(BPC):
                af = np.log(sv[b_, 2:130]) - ev[b_, 0] * ln2
                ab = np.log(sv[BPC + b_, 2:130]) - ev[BPC + b_, 0] * ln2
                ls = af[1:128] + ab[127:0:-1]
                fin = np.isfinite(ls)
                mm = ls[fin].max()
                loss[core * BPC + b_] = -(np.log(np.exp(ls[fin] - mm).sum()) + mm)
    return loss


def kernel(y_true, y_pred):
    nc = _build()
    in_maps = _host_tensors(y_true, y_pred)
    res = run_bass_kernel_spmd(nc, in_maps, list(range(NCORES)))
    souts = [np.asarray(res.results[i]["souts"]) for i in range(NCORES)]
    eouts = [np.asarray(res.results[i]["eouts"]) for i in range(NCORES)]
    loss = _combine(souts, eouts)
    return loss.astype(np.float32)[:, None]


# revision 17
# speedup vs baseline: 2.6069x; 1.2486x over previous
"""CTC loss (Keras ctc_batch_cost semantics) on 8 Trainium2 NeuronCores.

Strategy (v3)
-------------
Data-parallel over batch: each core takes 32 of the 256 sequences, and
runs the fwd chain (t=0..255) and the bwd chain (t=511..256, states
reversed) together as 64 rows of one transposed-layout DP.

The DP runs in PROBABILITY space: with (b,dir) on SBUF partitions and
the extended-label state s on the free dimension, one time step is

    S'[r, s] = q_j[r,s] * (S[r,s] + S[r,s-1]) + qg_j[r,s] * S[r,s-2]

where q = y_pred[., t, ext[s]] + eps (gathered emission probs) and
qg = q * skip-gate.  The state shifts are free-dim AP offsets (an
overlapping stride -1 view), so a step is 4 bf16 DVE instructions and
nothing else -- no matmuls, no PSUM, no log/exp in the serial chain.

fp32/bf16 range is handled by a flush every KF steps: the max of S
over a window around the wavefront diagonal (s ~ j/2) is rescaled to
2^BIAS by an exact power of two (exponent bit arithmetic on DVE), a
high cap protects runaway leader states, and the applied log2-scale
accumulates per row.  States that underflow relative to the window are
> e^-45 below every contributing path -- dropping them is harmless at
the 2e-2 tolerance (validated vs the f32 reference at ~6e-3 max rel
err over all 256 examples in numpy simulation of this exact bf16
arithmetic).

The q/qg tables are produced on device, overlapped with the DP: the
host supplies y transposed to [v, t] (and a time-reversed copy for the
bwd chains) in bf16; one-hot gather matmuls (eps folded in, the skip
gate folded into the qg columns) produce [t, 2, 128] tiles in PSUM, an
ACT/DVE copy moves them to SBUF bf16, and one SWDGE DMA per
(b,dir,chunk) scatters rows into the per-row [j, 2, 128] qcat layout.

Host combine in f64: loss = -logsumexp_s(alpha[s] + betahat[s]),
alpha = log(S_fwd) - E_fwd*log 2, exactly as the validated v1 combine.
"""

import sys

sys.path.insert(0, "/opt/trn_rl_repo")

from contextlib import ExitStack

import numpy as np
import ml_dtypes

import concourse.bass as bass
import concourse.tile as tile
from concourse import bacc, mybir
from concourse.ap import AP
from concourse.bass_utils import run_bass_kernel_spmd

bf16 = ml_dtypes.bfloat16

B, T, V, L = 256, 512, 256, 64
S = 2 * L + 1            # 129 extended states; chains keep 128 each
BLANK = V - 1
EPS = 1e-7
NCORES = 8
BPC = B // NCORES        # 32 examples per core
NJ = T // 2              # 256 time steps per chain
KF = 8                   # flush period
BIAS = 64                # flush rescales window max to 2^BIAS
WIN = 24                 # flush window half-width around the diagonal
CAPF = float(2.0 ** 101)
FP32 = mybir.dt.float32
BF16 = mybir.dt.bfloat16
I32 = mybir.dt.int32
ALU = mybir.AluOpType


def _kernel_body(ctx, tc, ytf_in, ytr_in, g_in, s_out, e_out):
    nc = tc.nc

    const_pool = ctx.enter_context(tc.tile_pool(name="const", bufs=1))
    g_pool = ctx.enter_context(tc.tile_pool(name="gmat", bufs=3))
    qcat_pool = ctx.enter_context(tc.tile_pool(name="qcat", bufs=1))
    yt_pool = ctx.enter_context(tc.tile_pool(name="yt", bufs=8))
    qs_pool = ctx.enter_context(tc.tile_pool(name="qs", bufs=10))
    psum_g = ctx.enter_context(tc.tile_pool(name="pg", bufs=8, space="PSUM"))
    work = ctx.enter_context(tc.tile_pool(name="work", bufs=4))

    # q tables: per row r=(d,b), per step j: [2, 128] (q | q*gate)
    qcat = qcat_pool.tile([64, NJ, 2, 128], BF16)

    # ---- production --------------------------------------------------
    # group ch covers j in [ch*128, (ch+1)*128): fwd rows consume ytf
    # t-chunk ch, bwd rows consume ytr t-chunk ch (already reversed).
    YB = 8                       # examples per yt slice DMA
    GB = 4                       # examples per gm slice DMA

    def produce_group(ch):
        _save = tc.cur_priority
        tc.cur_priority = _save + 1_000_000
        for b0 in range(0, BPC, GB):
            gm = g_pool.tile([128, GB, 2, 2, 256], BF16, tag="gm")
            ga = g_in[b0:b0 + GB]
            gsrc = AP(ga.tensor, ga.offset,
                      [[1024, 128], [128 * 1024, GB], [1, 1024]])
            nc.sync.dma_start(gm[:], gsrc)
            yts = {}
            for d_ in range(2):
                if b0 % YB == 0:
                    for h in range(2):
                        yt_ = yt_pool.tile([128, YB, 128], BF16,
                                           tag=f"yt{d_}{h}")
                        ya = (ytf_in if d_ == 0 else ytr_in)
                        off = (h * 128 * BPC * T + b0 * T + ch * 128)
                        src = AP(ya.tensor, off,
                                 [[BPC * T, 128], [T, YB], [1, 128]])
                        if ch == 0 and d_ == 0:
                            nc.scalar.dma_start(yt_[:], src)
                        else:
                            nc.sync.dma_start(yt_[:], src)
                        yts[(d_, h)] = yt_
                        produce_group.yts[(d_, h)] = yt_
                else:
                    yts = produce_group.yts
            for bi in range(GB):
                b_ = b0 + bi
                for d_ in range(2):
                    yth = produce_group.yts
                    gps = psum_g.tile([128, 256], FP32, tag="gps")
                    for h in range(2):
                        nc.tensor.matmul(gps[:],
                                         yth[(d_, h)][:, b_ % YB, :],
                                         gm[:, bi, d_, h, :],
                                         start=(h == 0), stop=(h == 1))
                    qs = qs_pool.tile([128, 2, 128], BF16, tag="qs")
                    if ch == 0 and (b_ + d_) % 2 == 0:
                        nc.vector.tensor_copy(qs[:], gps[:])
                    else:
                        nc.scalar.copy(qs[:], gps[:])
                    r_ = d_ * BPC + b_
                    a = qcat[r_:r_ + 1, ch * 128:(ch + 1) * 128, :, :]
                    dst = AP(a.tensor, a.offset,
                             [list(a.ap[0]), [256, 128], [1, 256]])
                    if ch == 0 and d_ == 1:
                        nc.sync.dma_start(dst, qs[:])
                    else:
                        nc.gpsimd.dma_start(dst, qs[:])
        tc.cur_priority = _save

    produce_group.yts = {}

    produce_group(0)

    # ---- DP chain ----------------------------------------------------
    SA = const_pool.tile([64, 130], BF16)
    SB = const_pool.tile([64, 130], BF16)
    eacc = const_pool.tile([64, 1], FP32)
    nc.vector.memset(SA[:], 0.0)
    nc.vector.memset(SB[:], 0.0)
    nc.vector.memset(eacc[:], 0.0)
    nc.vector.tensor_copy(SA[:, 2:4], qcat[:, 0, 0, 0:2])

    cur, nxt = SA, SB
    for j in range(1, NJ):
        if j == 64:
            produce_group(1)
        qj = qcat[:, j]
        q0b = qj[:, 0, :].unsqueeze(1).broadcast_to([64, 2, 128])
        sap = cur[:]
        s2v = AP(sap.tensor, sap.offset + 2,
                 [list(sap.ap[0]), [-1, 2], [1, 128]])
        m = work.tile([64, 3, 128], BF16, tag="m")
        nc.vector.tensor_mul(m[:, 0:2, :], q0b, s2v)
        nc.vector.tensor_mul(m[:, 2, :], qj[:, 1, :], cur[:, 0:128])
        u = work.tile([64, 128], BF16, tag="u")
        nc.vector.tensor_add(u[:], m[:, 0, :], m[:, 1, :])
        nc.vector.tensor_add(nxt[:, 2:130], u[:], m[:, 2, :])
        cur, nxt = nxt, cur

        if j % KF == 0 and j < NJ - 1:
            s0 = j // 2
            lo, hi = max(0, s0 - WIN), min(128, s0 + WIN + 1)
            wm32 = work.tile([64, 1], FP32, tag="wm32")
            nc.vector.tensor_reduce(wm32[:], cur[:, 2 + lo:2 + hi],
                                    axis=mybir.AxisListType.X, op=ALU.max)
            t1 = work.tile([64, 1], I32, tag="t1")
            nc.vector.tensor_scalar(t1[:], wm32[:].bitcast(I32), 23, -1,
                                    op0=ALU.logical_shift_right,
                                    op1=ALU.bitwise_xor)
            f = work.tile([64, 1], I32, tag="f")
            nc.vector.tensor_scalar(f[:], t1[:], BIAS + 255, 254,
                                    op0=ALU.add, op1=ALU.min)
            nc.vector.scalar_tensor_tensor(eacc[:], f[:], -127.0, eacc[:],
                                           ALU.add, ALU.add)
            sc_i = work.tile([64, 1], I32, tag="sci")
            nc.vector.tensor_scalar(sc_i[:], f[:], 23, None,
                                    op0=ALU.logical_shift_left)
            nc.vector.tensor_scalar(nxt[:], cur[:], sc_i[:].bitcast(FP32), CAPF,
                                    op0=ALU.mult, op1=ALU.min)
            cur, nxt = nxt, cur

    nc.sync.dma_start(s_out, cur[:])
    nc.sync.dma_start(e_out, eacc[:])


_CACHED = None


def _build():
    global _CACHED
    if _CACHED is not None:
        return _CACHED
    nc = bacc.Bacc("TRN2", target_bir_lowering=False, debug=False,
                   num_devices=NCORES)
    ytf_in = nc.dram_tensor("ytf", [BPC, 2, 128, T], BF16,
                            kind="ExternalInput").ap()
    ytr_in = nc.dram_tensor("ytr", [BPC, 2, 128, T], BF16,
                            kind="ExternalInput").ap()
    g_in = nc.dram_tensor("g", [BPC, 128, 2, 2, 256], BF16,
                          kind="ExternalInput").ap()
    s_out = nc.dram_tensor("souts", [64, 130], BF16, kind="ExternalOutput").ap()
    e_out = nc.dram_tensor("eouts", [64, 1], FP32, kind="ExternalOutput").ap()

    with tile.TileContext(nc) as tc:
        with ExitStack() as ctx:
            _kernel_body(ctx, tc, ytf_in, ytr_in, g_in, s_out, e_out)
    nc.compile()
    _CACHED = nc
    return nc


def _host_tensors(y_true, y_pred):
    """Per-core input dicts. Host does layout only: y transposed to
    [v,t] bf16 (plus a time-reversed copy) and one-hot gather matrices."""
    y_true = np.asarray(y_true)
    y_pred = np.asarray(y_pred, dtype=np.float32)

    in_maps = []
    for core in range(NCORES):
        bs = slice(core * BPC, (core + 1) * BPC)
        yt_c = y_true[bs]
        # [b, t, v] -> [b, h, v128, t] transposed bf16
        ytb = np.ascontiguousarray(
            y_pred[bs].transpose(0, 2, 1).reshape(BPC, 2, 128, T)).astype(bf16)
        ytr = np.ascontiguousarray(ytb[:, :, :, ::-1])
        g = np.zeros((BPC, 128, 2, 2, 256), np.float32)
        for b_ in range(BPC):
            ext = np.full(S, BLANK, dtype=np.int64)
            ext[1::2] = yt_c[b_]
            extm2 = np.concatenate([np.full(2, -1, dtype=np.int64), ext[:-2]])
            skip = ((ext != BLANK) & (ext != extm2)).astype(np.float32)
            # fwd (d=0): col s = 0..127 from ext[s]
            gf = np.zeros(128, np.float32)
            gf[2:] = skip[2:128]
            vf = ext[0:128]
            # bwd (d=1): col r = 0..127 from ext[128-r]
            gb = np.zeros(128, np.float32)
            rarr = np.arange(2, 128)
            gb[rarr] = skip[130 - rarr]
            vb = ext[128 - np.arange(128)]
            for d_, vv, gg in ((0, vf, gf), (1, vb, gb)):
                for s_ in range(128):
                    v = int(vv[s_])
                    # q column: onehot + eps on every v row
                    g[b_, v % 128, d_, v // 128, s_] += 1.0
                    g[b_, :, d_, :, s_] += EPS
                    # qg column: (onehot + eps) * gate
                    if gg[s_] > 0:
                        g[b_, v % 128, d_, v // 128, 128 + s_] += 1.0
                        g[b_, :, d_, :, 128 + s_] += EPS
        in_maps.append({
            "ytf": ytb,
            "ytr": ytr,
            "g": g.astype(bf16),
        })
    return in_maps


def _combine(souts, eouts):
    """Host f64 combine: loss[b] = -logsumexp_s(alpha[s] + betahat[s])."""
    ln2 = np.log(2.0)
    loss = np.zeros(B, dtype=np.float64)
    with np.errstate(divide="ignore"):
        for core in range(NCORES):
            sv = souts[core].astype(np.float64)
            ev = eouts[core].astype(np.float64)
            for b_ in range(BPC):
                af = np.log(sv[b_, 2:130]) - ev[b_, 0] * ln2
                ab = np.log(sv[BPC + b_, 2:130]) - ev[BPC + b_, 0] * ln2
                ls = af[1:128] + ab[127:0:-1]
                fin = np.isfinite(ls)
                mm = ls[fin].max()
                loss[core * BPC + b_] = -(np.log(np.exp(ls[fin] - mm).sum()) + mm)
    return loss


def kernel(y_true, y_pred):
    nc = _build()
    in_maps = _host_tensors(y_true, y_pred)
    res = run_bass_kernel_spmd(nc, in_maps, list(range(NCORES)))
    souts = [np.asarray(res.results[i]["souts"]) for i in range(NCORES)]
    eouts = [np.asarray(res.results[i]["eouts"]) for i in range(NCORES)]
    loss = _combine(souts, eouts)
    return loss.astype(np.float32)[:, None]


# revision 20
# speedup vs baseline: 2.8601x; 1.0971x over previous
"""CTC loss (Keras ctc_batch_cost semantics) on 8 Trainium2 NeuronCores.

Strategy (v3)
-------------
Data-parallel over batch: each core takes 32 of the 256 sequences, and
runs the fwd chain (t=0..255) and the bwd chain (t=511..256, states
reversed) together as 64 rows of one transposed-layout DP.

The DP runs in PROBABILITY space: with (b,dir) on SBUF partitions and
the extended-label state s on the free dimension, one time step is

    S'[r, s] = q_j[r,s] * (S[r,s] + S[r,s-1]) + qg_j[r,s] * S[r,s-2]

where q = y_pred[., t, ext[s]] + eps (gathered emission probs) and
qg = q * skip-gate.  The state shifts are free-dim AP offsets (an
overlapping stride -1 view), so a step is 4 bf16 DVE instructions and
nothing else -- no matmuls, no PSUM, no log/exp in the serial chain.

fp32/bf16 range is handled by a flush every KF steps: the max of S
over a window around the wavefront diagonal (s ~ j/2) is rescaled to
2^BIAS by an exact power of two (exponent bit arithmetic on DVE), a
high cap protects runaway leader states, and the applied log2-scale
accumulates per row.  States that underflow relative to the window are
> e^-45 below every contributing path -- dropping them is harmless at
the 2e-2 tolerance (validated vs the f32 reference at ~6e-3 max rel
err over all 256 examples in numpy simulation of this exact bf16
arithmetic).

The q/qg tables are produced on device, overlapped with the DP: the
host supplies y transposed to [v, t] (and a time-reversed copy for the
bwd chains) in bf16; one-hot gather matmuls (eps folded in, the skip
gate folded into the qg columns) produce [t, 2, 128] tiles in PSUM, an
ACT/DVE copy moves them to SBUF bf16, and one SWDGE DMA per
(b,dir,chunk) scatters rows into the per-row [j, 2, 128] qcat layout.

Host combine in f64: loss = -logsumexp_s(alpha[s] + betahat[s]),
alpha = log(S_fwd) - E_fwd*log 2, exactly as the validated v1 combine.
"""

import sys

sys.path.insert(0, "/opt/trn_rl_repo")

from contextlib import ExitStack

import numpy as np
import ml_dtypes

import concourse.bass as bass
import concourse.tile as tile
from concourse import bacc, mybir
from concourse.ap import AP
from concourse.bass_utils import run_bass_kernel_spmd

bf16 = ml_dtypes.bfloat16

B, T, V, L = 256, 512, 256, 64
S = 2 * L + 1            # 129 extended states; chains keep 128 each
BLANK = V - 1
EPS = 1e-7
NCORES = 8
BPC = B // NCORES        # 32 examples per core
NJ = T // 2              # 256 time steps per chain
KF = 8                   # flush period
BIAS = 64                # flush rescales window max to 2^BIAS
WIN = 24                 # flush window half-width around the diagonal
CAPF = float(2.0 ** 101)
FP32 = mybir.dt.float32
BF16 = mybir.dt.bfloat16
I32 = mybir.dt.int32
ALU = mybir.AluOpType


def _kernel_body(ctx, tc, ytf_in, ytr_in, g_in, s_out, e_out):
    nc = tc.nc

    const_pool = ctx.enter_context(tc.tile_pool(name="const", bufs=1))
    g_pool = ctx.enter_context(tc.tile_pool(name="gmat", bufs=3))
    qcat_pool = ctx.enter_context(tc.tile_pool(name="qcat", bufs=1))
    yt_pool = ctx.enter_context(tc.tile_pool(name="yt", bufs=3))
    qs_pool = ctx.enter_context(tc.tile_pool(name="qs", bufs=10))
    psum_g = ctx.enter_context(tc.tile_pool(name="pg", bufs=8, space="PSUM"))
    work = ctx.enter_context(tc.tile_pool(name="work", bufs=4))

    # q tables: per row r=(d,b), per step j: [2, 128] (q | q*gate)
    qcat = qcat_pool.tile([64, NJ, 2, 128], BF16)

    # ---- production --------------------------------------------------
    # group ch covers j in [ch*128, (ch+1)*128): fwd rows consume ytf
    # t-chunk ch, bwd rows consume ytr t-chunk ch (already reversed).
    YB = 8                       # examples per yt slice DMA
    GB = 4                       # examples per gm slice DMA

    def produce_group(ch):
        _save = tc.cur_priority
        tc.cur_priority = _save + 1_000_000
        for b0 in range(0, BPC, GB):
            gm = g_pool.tile([128, GB, 2, 2, 256], BF16, tag="gm")
            ga = g_in[b0:b0 + GB]
            gsrc = AP(ga.tensor, ga.offset,
                      [[1024, 128], [128 * 1024, GB], [1, 1024]])
            nc.sync.dma_start(gm[:], gsrc)
            yts = {}
            for d_ in range(2):
                if b0 % YB == 0:
                    for h in range(2):
                        yt_ = yt_pool.tile([128, YB, 128], BF16,
                                           tag=f"yt{d_}{h}")
                        ya = (ytf_in if d_ == 0 else ytr_in)
                        off = (h * 128 * BPC * T + b0 * T + ch * 128)
                        src = AP(ya.tensor, off,
                                 [[BPC * T, 128], [T, YB], [1, 128]])
                        if ch == 0 and d_ == 0:
                            nc.scalar.dma_start(yt_[:], src)
                        else:
                            nc.sync.dma_start(yt_[:], src)
                        yts[(d_, h)] = yt_
                        produce_group.yts[(d_, h)] = yt_
                else:
                    yts = produce_group.yts
            for bi in range(GB):
                b_ = b0 + bi
                for d_ in range(2):
                    yth = produce_group.yts
                    gps = psum_g.tile([128, 256], FP32, tag="gps")
                    for h in range(2):
                        nc.tensor.matmul(gps[:],
                                         yth[(d_, h)][:, b_ % YB, :],
                                         gm[:, bi, d_, h, :],
                                         start=(h == 0), stop=(h == 1))
                    qs = qs_pool.tile([128, 2, 128], BF16, tag="qs")
                    if ch == 0 and (b_ + d_) % 2 == 0:
                        nc.vector.tensor_copy(qs[:], gps[:])
                    else:
                        nc.scalar.copy(qs[:], gps[:])
                    r_ = d_ * BPC + b_
                    a = qcat[r_:r_ + 1, ch * 128:(ch + 1) * 128, :, :]
                    dst = AP(a.tensor, a.offset,
                             [list(a.ap[0]), [256, 128], [1, 256]])
                    if ch == 0 and d_ == 1:
                        nc.sync.dma_start(dst, qs[:])
                    else:
                        nc.gpsimd.dma_start(dst, qs[:])
        tc.cur_priority = _save

    produce_group.yts = {}

    produce_group(0)

    # ---- DP chain ----------------------------------------------------
    SA = const_pool.tile([64, 130], BF16)
    SB = const_pool.tile([64, 130], BF16)
    eacc = const_pool.tile([64, 1], FP32)
    nc.vector.memset(SA[:], 0.0)
    nc.vector.memset(SB[:], 0.0)
    nc.vector.memset(eacc[:], 0.0)
    nc.vector.tensor_copy(SA[:, 2:4], qcat[:, 0, 0, 0:2])

    cur, nxt = SA, SB
    for j in range(1, NJ):
        if j == 64:
            produce_group(1)
        w = min(128, 2 * j + 2)   # wavefront: states s >= 2j+2 are still 0
        qj = qcat[:, j]
        q0b = qj[:, 0, 0:w].unsqueeze(1).broadcast_to([64, 2, w])
        sap = cur[:]
        s2v = AP(sap.tensor, sap.offset + 2,
                 [list(sap.ap[0]), [-1, 2], [1, w]])
        m = work.tile([64, 3, 128], BF16, tag="m")
        nc.vector.tensor_mul(m[:, 0:2, 0:w], q0b, s2v)
        nc.vector.tensor_mul(m[:, 2, 0:w], qj[:, 1, 0:w], cur[:, 0:w])
        u = work.tile([64, 128], BF16, tag="u")
        nc.vector.tensor_add(u[:, 0:w], m[:, 0, 0:w], m[:, 1, 0:w])
        nc.vector.tensor_add(nxt[:, 2:2 + w], u[:, 0:w], m[:, 2, 0:w])
        cur, nxt = nxt, cur

        if j % KF == 0 and j < NJ - 1:
            s0 = j // 2
            lo, hi = max(0, s0 - WIN), min(128, s0 + WIN + 1)
            wm32 = work.tile([64, 1], FP32, tag="wm32")
            nc.vector.tensor_reduce(wm32[:], cur[:, 2 + lo:2 + hi],
                                    axis=mybir.AxisListType.X, op=ALU.max)
            t1 = work.tile([64, 1], I32, tag="t1")
            nc.vector.tensor_scalar(t1[:], wm32[:].bitcast(I32), 23, -1,
                                    op0=ALU.logical_shift_right,
                                    op1=ALU.bitwise_xor)
            f = work.tile([64, 1], I32, tag="f")
            nc.vector.tensor_scalar(f[:], t1[:], BIAS + 255, 254,
                                    op0=ALU.add, op1=ALU.min)
            nc.vector.scalar_tensor_tensor(eacc[:], f[:], -127.0, eacc[:],
                                           ALU.add, ALU.add)
            sc_i = work.tile([64, 1], I32, tag="sci")
            nc.vector.tensor_scalar(sc_i[:], f[:], 23, None,
                                    op0=ALU.logical_shift_left)
            nc.vector.tensor_scalar(nxt[:], cur[:], sc_i[:].bitcast(FP32), CAPF,
                                    op0=ALU.mult, op1=ALU.min)
            cur, nxt = nxt, cur

    nc.sync.dma_start(s_out, cur[:])
    nc.sync.dma_start(e_out, eacc[:])


_CACHED = None


def _build():
    global _CACHED
    if _CACHED is not None:
        return _CACHED
    nc = bacc.Bacc("TRN2", target_bir_lowering=False, debug=False,
                   num_devices=NCORES)
    ytf_in = nc.dram_tensor("ytf", [2, 128, BPC, T], BF16,
                            kind="ExternalInput").ap()
    ytr_in = nc.dram_tensor("ytr", [2, 128, BPC, T], BF16,
                            kind="ExternalInput").ap()
    g_in = nc.dram_tensor("g", [BPC, 128, 2, 2, 256], BF16,
                          kind="ExternalInput").ap()
    s_out = nc.dram_tensor("souts", [64, 130], BF16, kind="ExternalOutput").ap()
    e_out = nc.dram_tensor("eouts", [64, 1], FP32, kind="ExternalOutput").ap()

    with tile.TileContext(nc) as tc:
        with ExitStack() as ctx:
            _kernel_body(ctx, tc, ytf_in, ytr_in, g_in, s_out, e_out)
    nc.compile()
    _CACHED = nc
    return nc


def _host_tensors(y_true, y_pred):
    """Per-core input dicts. Host does layout only: y transposed to
    [v,t] bf16 (plus a time-reversed copy) and one-hot gather matrices."""
    y_true = np.asarray(y_true)
    y_pred = np.asarray(y_pred, dtype=np.float32)

    in_maps = []
    for core in range(NCORES):
        bs = slice(core * BPC, (core + 1) * BPC)
        yt_c = y_true[bs]
        # [b, t, v] -> [h, v128, b, t] transposed bf16
        ytb = np.ascontiguousarray(
            y_pred[bs].transpose(2, 0, 1).reshape(2, 128, BPC, T)).astype(bf16)
        ytr = np.ascontiguousarray(ytb[:, :, :, ::-1])
        g = np.zeros((BPC, 128, 2, 2, 256), np.float32)
        for b_ in range(BPC):
            ext = np.full(S, BLANK, dtype=np.int64)
            ext[1::2] = yt_c[b_]
            extm2 = np.concatenate([np.full(2, -1, dtype=np.int64), ext[:-2]])
            skip = ((ext != BLANK) & (ext != extm2)).astype(np.float32)
            # fwd (d=0): col s = 0..127 from ext[s]
            gf = np.zeros(128, np.float32)
            gf[2:] = skip[2:128]
            vf = ext[0:128]
            # bwd (d=1): col r = 0..127 from ext[128-r]
            gb = np.zeros(128, np.float32)
            rarr = np.arange(2, 128)
            gb[rarr] = skip[130 - rarr]
            vb = ext[128 - np.arange(128)]
            for d_, vv, gg in ((0, vf, gf), (1, vb, gb)):
                for s_ in range(128):
                    v = int(vv[s_])
                    # q column: onehot + eps on every v row
                    g[b_, v % 128, d_, v // 128, s_] += 1.0
                    g[b_, :, d_, :, s_] += EPS
                    # qg column: (onehot + eps) * gate
                    if gg[s_] > 0:
                        g[b_, v % 128, d_, v // 128, 128 + s_] += 1.0
                        g[b_, :, d_, :, 128 + s_] += EPS
        in_maps.append({
            "ytf": ytb,
            "ytr": ytr,
            "g": g.astype(bf16),
        })
    return in_maps


def _combine(souts, eouts):
    """Host f64 combine: loss[b] = -logsumexp_s(alpha[s] + betahat[s])."""
    ln2 = np.log(2.0)
    loss = np.zeros(B, dtype=np.float64)
    with np.errstate(divide="ignore"):
        for core in range(NCORES):
            sv = souts[core].astype(np.float64)
            ev = eouts[core].astype(np.float64)
            for b_ in range(BPC):
                af = np.log(sv[b_, 2:130]) - ev[b_, 0] * ln2
                ab = np.log(sv[BPC + b_, 2:130]) - ev[BPC + b_, 0] * ln2
                ls = af[1:128] + ab[127:0:-1]
                fin = np.isfinite(ls)
                mm = ls[fin].max()
                loss[core * BPC + b_] = -(np.log(np.exp(ls[fin] - mm).sum()) + mm)
    return loss


def kernel(y_true, y_pred):
    nc = _build()
    in_maps = _host_tensors(y_true, y_pred)
    res = run_bass_kernel_spmd(nc, in_maps, list(range(NCORES)))
    souts = [np.asarray(res.results[i]["souts"]) for i in range(NCORES)]
    eouts = [np.asarray(res.results[i]["eouts"]) for i in range(NCORES)]
    loss = _combine(souts, eouts)
    return loss.astype(np.float32)[:, None]


# revision 24
# speedup vs baseline: 2.8945x; 1.0120x over previous
"""CTC loss (Keras ctc_batch_cost semantics) on 8 Trainium2 NeuronCores.

Strategy (v3)
-------------
Data-parallel over batch: each core takes 32 of the 256 sequences, and
runs the fwd chain (t=0..255) and the bwd chain (t=511..256, states
reversed) together as 64 rows of one transposed-layout DP.

The DP runs in PROBABILITY space: with (b,dir) on SBUF partitions and
the extended-label state s on the free dimension, one time step is

    S'[r, s] = q_j[r,s] * (S[r,s] + S[r,s-1]) + qg_j[r,s] * S[r,s-2]

where q = y_pred[., t, ext[s]] + eps (gathered emission probs) and
qg = q * skip-gate.  The state shifts are free-dim AP offsets (an
overlapping stride -1 view), so a step is 4 bf16 DVE instructions and
nothing else -- no matmuls, no PSUM, no log/exp in the serial chain.

fp32/bf16 range is handled by a flush every KF steps: the max of S
over a window around the wavefront diagonal (s ~ j/2) is rescaled to
2^BIAS by an exact power of two (exponent bit arithmetic on DVE), a
high cap protects runaway leader states, and the applied log2-scale
accumulates per row.  States that underflow relative to the window are
> e^-45 below every contributing path -- dropping them is harmless at
the 2e-2 tolerance (validated vs the f32 reference at ~6e-3 max rel
err over all 256 examples in numpy simulation of this exact bf16
arithmetic).

The q/qg tables are produced on device, overlapped with the DP: the
host supplies y transposed to [v, t] (and a time-reversed copy for the
bwd chains) in bf16; one-hot gather matmuls (eps folded in, the skip
gate folded into the qg columns) produce [t, 2, 128] tiles in PSUM, an
ACT/DVE copy moves them to SBUF bf16, and one SWDGE DMA per
(b,dir,chunk) scatters rows into the per-row [j, 2, 128] qcat layout.

Host combine in f64: loss = -logsumexp_s(alpha[s] + betahat[s]),
alpha = log(S_fwd) - E_fwd*log 2, exactly as the validated v1 combine.
"""

import sys

sys.path.insert(0, "/opt/trn_rl_repo")

from contextlib import ExitStack

import numpy as np
import ml_dtypes

import concourse.bass as bass
import concourse.tile as tile
from concourse import bacc, mybir
from concourse.ap import AP
from concourse.bass_utils import run_bass_kernel_spmd

bf16 = ml_dtypes.bfloat16

B, T, V, L = 256, 512, 256, 64
S = 2 * L + 1            # 129 extended states; chains keep 128 each
BLANK = V - 1
EPS = 1e-7
NCORES = 8
BPC = B // NCORES        # 32 examples per core
NJ = T // 2              # 256 time steps per chain
KF = 8                   # flush period
BIAS = 64                # flush rescales window max to 2^BIAS
WIN = 24                 # flush window half-width around the diagonal
CAPF = float(2.0 ** 101)
FP32 = mybir.dt.float32
BF16 = mybir.dt.bfloat16
I32 = mybir.dt.int32
ALU = mybir.AluOpType


def _kernel_body(ctx, tc, ytf_in, ytr_in, g_in, s_out, e_out):
    nc = tc.nc

    const_pool = ctx.enter_context(tc.tile_pool(name="const", bufs=1))
    g_pool = ctx.enter_context(tc.tile_pool(name="gmat", bufs=2))
    qcat_pool = ctx.enter_context(tc.tile_pool(name="qcat", bufs=1))
    yt_pool = ctx.enter_context(tc.tile_pool(name="yt", bufs=3))
    qs_pool = ctx.enter_context(tc.tile_pool(name="qs", bufs=10))
    psum_g = ctx.enter_context(tc.tile_pool(name="pg", bufs=8, space="PSUM"))
    work = ctx.enter_context(tc.tile_pool(name="work", bufs=4))

    # q tables: per row r=(d,b), per step j: [2, 128] (q | q*gate)
    qcat = qcat_pool.tile([64, NJ, 2, 128], BF16)

    # ---- production --------------------------------------------------
    # group ch covers j in [ch*128, (ch+1)*128): fwd rows consume ytf
    # t-chunk ch, bwd rows consume ytr t-chunk ch (already reversed).
    YB = 8                       # examples per yt slice DMA
    GB = 8                       # examples per gm slice DMA

    def produce_group(ch):
        _save = tc.cur_priority
        tc.cur_priority = _save + 1_000_000
        for b0 in range(0, BPC, GB):
            gm = g_pool.tile([128, GB, 2, 2, 256], BF16, tag="gm")
            ga = g_in[b0:b0 + GB]
            gsrc = AP(ga.tensor, ga.offset,
                      [[1024, 128], [128 * 1024, GB], [1, 1024]])
            nc.sync.dma_start(gm[:], gsrc)
            yts = {}
            for d_ in range(2):
                if b0 % YB == 0:
                    for h in range(2):
                        yt_ = yt_pool.tile([128, YB, 128], BF16,
                                           tag=f"yt{d_}{h}")
                        ya = (ytf_in if d_ == 0 else ytr_in)
                        off = (h * 128 * BPC * T + b0 * T + ch * 128)
                        src = AP(ya.tensor, off,
                                 [[BPC * T, 128], [T, YB], [1, 128]])
                        if ch == 0 and d_ == 0:
                            nc.scalar.dma_start(yt_[:], src)
                        else:
                            nc.sync.dma_start(yt_[:], src)
                        yts[(d_, h)] = yt_
                        produce_group.yts[(d_, h)] = yt_
                else:
                    yts = produce_group.yts
            for bi in range(GB):
                b_ = b0 + bi
                for d_ in range(2):
                    yth = produce_group.yts
                    gps = psum_g.tile([128, 256], FP32, tag="gps")
                    for h in range(2):
                        nc.tensor.matmul(gps[:],
                                         yth[(d_, h)][:, b_ % YB, :],
                                         gm[:, bi, d_, h, :],
                                         start=(h == 0), stop=(h == 1))
                    qs = qs_pool.tile([128, 2, 128], BF16, tag="qs")
                    if ch == 0 and (b_ + d_) % 2 == 0:
                        nc.vector.tensor_copy(qs[:], gps[:])
                    else:
                        nc.scalar.copy(qs[:], gps[:])
                    r_ = d_ * BPC + b_
                    a = qcat[r_:r_ + 1, ch * 128:(ch + 1) * 128, :, :]
                    dst = AP(a.tensor, a.offset,
                             [list(a.ap[0]), [256, 128], [1, 256]])
                    if ch == 0 and d_ == 1:
                        nc.sync.dma_start(dst, qs[:])
                    else:
                        nc.gpsimd.dma_start(dst, qs[:])
        tc.cur_priority = _save

    produce_group.yts = {}

    produce_group(0)

    # ---- DP chain ----------------------------------------------------
    SA = const_pool.tile([64, 130], BF16)
    SB = const_pool.tile([64, 130], BF16)
    eacc = const_pool.tile([64, 1], FP32)
    nc.vector.memset(SA[:], 0.0)
    nc.vector.memset(SB[:], 0.0)
    nc.vector.memset(eacc[:], 0.0)
    nc.vector.tensor_copy(SA[:, 2:4], qcat[:, 0, 0, 0:2])

    cur, nxt = SA, SB
    for j in range(1, NJ):
        if j == 64:
            produce_group(1)
        w = min(128, 2 * j + 2)   # wavefront: states s >= 2j+2 are still 0
        qj = qcat[:, j]
        q0b = qj[:, 0, 0:w].unsqueeze(1).broadcast_to([64, 2, w])
        sap = cur[:]
        s2v = AP(sap.tensor, sap.offset + 2,
                 [list(sap.ap[0]), [-1, 2], [1, w]])
        m = work.tile([64, 3, 128], BF16, tag="m")
        nc.vector.tensor_mul(m[:, 0:2, 0:w], q0b, s2v)
        nc.vector.tensor_mul(m[:, 2, 0:w], qj[:, 1, 0:w], cur[:, 0:w])
        u = work.tile([64, 128], BF16, tag="u")
        nc.vector.tensor_add(u[:, 0:w], m[:, 0, 0:w], m[:, 1, 0:w])
        nc.vector.tensor_add(nxt[:, 2:2 + w], u[:, 0:w], m[:, 2, 0:w])
        cur, nxt = nxt, cur

        if j % KF == 0 and j < NJ - 1:
            s0 = j // 2
            lo, hi = max(0, s0 - WIN), min(128, s0 + WIN + 1)
            wm32 = work.tile([64, 1], FP32, tag="wm32")
            nc.vector.tensor_reduce(wm32[:], cur[:, 2 + lo:2 + hi],
                                    axis=mybir.AxisListType.X, op=ALU.max)
            t1 = work.tile([64, 1], I32, tag="t1")
            nc.vector.tensor_scalar(t1[:], wm32[:].bitcast(I32), 23, -1,
                                    op0=ALU.logical_shift_right,
                                    op1=ALU.bitwise_xor)
            f = work.tile([64, 1], I32, tag="f")
            nc.vector.tensor_scalar(f[:], t1[:], BIAS + 255, 254,
                                    op0=ALU.add, op1=ALU.min)
            nc.vector.scalar_tensor_tensor(eacc[:], f[:], -127.0, eacc[:],
                                           ALU.add, ALU.add)
            sc_i = work.tile([64, 1], I32, tag="sci")
            nc.vector.tensor_scalar(sc_i[:], f[:], 23, None,
                                    op0=ALU.logical_shift_left)
            nc.vector.tensor_scalar(nxt[:], cur[:], sc_i[:].bitcast(FP32), CAPF,
                                    op0=ALU.mult, op1=ALU.min)
            cur, nxt = nxt, cur

    nc.sync.dma_start(s_out, cur[:])
    nc.sync.dma_start(e_out, eacc[:])


_CACHED = None


def _build():
    global _CACHED
    if _CACHED is not None:
        return _CACHED
    nc = bacc.Bacc("TRN2", target_bir_lowering=False, debug=False,
                   num_devices=NCORES)
    ytf_in = nc.dram_tensor("ytf", [2, 128, BPC, T], BF16,
                            kind="ExternalInput").ap()
    ytr_in = nc.dram_tensor("ytr", [2, 128, BPC, T], BF16,
                            kind="ExternalInput").ap()
    g_in = nc.dram_tensor("g", [BPC, 128, 2, 2, 256], BF16,
                          kind="ExternalInput").ap()
    s_out = nc.dram_tensor("souts", [64, 130], BF16, kind="ExternalOutput").ap()
    e_out = nc.dram_tensor("eouts", [64, 1], FP32, kind="ExternalOutput").ap()

    with tile.TileContext(nc) as tc:
        with ExitStack() as ctx:
            _kernel_body(ctx, tc, ytf_in, ytr_in, g_in, s_out, e_out)
    nc.compile()
    _CACHED = nc
    return nc


def _host_tensors(y_true, y_pred):
    """Per-core input dicts. Host does layout only: y transposed to
    [v,t] bf16 (plus a time-reversed copy) and one-hot gather matrices."""
    y_true = np.asarray(y_true)
    y_pred = np.asarray(y_pred, dtype=np.float32)

    in_maps = []
    for core in range(NCORES):
        bs = slice(core * BPC, (core + 1) * BPC)
        yt_c = y_true[bs]
        # [b, t, v] -> [h, v128, b, t] transposed bf16
        ytb = np.ascontiguousarray(
            y_pred[bs].transpose(2, 0, 1).reshape(2, 128, BPC, T)).astype(bf16)
        ytr = np.ascontiguousarray(ytb[:, :, :, ::-1])
        g = np.zeros((BPC, 128, 2, 2, 256), np.float32)
        for b_ in range(BPC):
            ext = np.full(S, BLANK, dtype=np.int64)
            ext[1::2] = yt_c[b_]
            extm2 = np.concatenate([np.full(2, -1, dtype=np.int64), ext[:-2]])
            skip = ((ext != BLANK) & (ext != extm2)).astype(np.float32)
            # fwd (d=0): col s = 0..127 from ext[s]
            gf = np.zeros(128, np.float32)
            gf[2:] = skip[2:128]
            vf = ext[0:128]
            # bwd (d=1): col r = 0..127 from ext[128-r]
            gb = np.zeros(128, np.float32)
            rarr = np.arange(2, 128)
            gb[rarr] = skip[130 - rarr]
            vb = ext[128 - np.arange(128)]
            for d_, vv, gg in ((0, vf, gf), (1, vb, gb)):
                for s_ in range(128):
                    v = int(vv[s_])
                    # q column: onehot + eps on every v row
                    g[b_, v % 128, d_, v // 128, s_] += 1.0
                    g[b_, :, d_, :, s_] += EPS
                    # qg column: (onehot + eps) * gate
                    if gg[s_] > 0:
                        g[b_, v % 128, d_, v // 128, 128 + s_] += 1.0
                        g[b_, :, d_, :, 128 + s_] += EPS
        in_maps.append({
            "ytf": ytb,
            "ytr": ytr,
            "g": g.astype(bf16),
        })
    return in_maps


def _combine(souts, eouts):
    """Host f64 combine: loss[b] = -logsumexp_s(alpha[s] + betahat[s])."""
    ln2 = np.log(2.0)
    loss = np.zeros(B, dtype=np.float64)
    with np.errstate(divide="ignore"):
        for core in range(NCORES):
            sv = souts[core].astype(np.float64)
            ev = eouts[core].astype(np.float64)
            for b_ in range(BPC):
                af = np.log(sv[b_, 2:130]) - ev[b_, 0] * ln2
                ab = np.log(sv[BPC + b_, 2:130]) - ev[BPC + b_, 0] * ln2
                ls = af[1:128] + ab[127:0:-1]
                fin = np.isfinite(ls)
                mm = ls[fin].max()
                loss[core * BPC + b_] = -(np.log(np.exp(ls[fin] - mm).sum()) + mm)
    return loss


def kernel(y_true, y_pred):
    nc = _build()
    in_maps = _host_tensors(y_true, y_pred)
    res = run_bass_kernel_spmd(nc, in_maps, list(range(NCORES)))
    souts = [np.asarray(res.results[i]["souts"]) for i in range(NCORES)]
    eouts = [np.asarray(res.results[i]["eouts"]) for i in range(NCORES)]
    loss = _combine(souts, eouts)
    return loss.astype(np.float32)[:, None]


# revision 25
# speedup vs baseline: 2.9767x; 1.0284x over previous
"""CTC loss (Keras ctc_batch_cost semantics) on 8 Trainium2 NeuronCores.

Strategy (v3)
-------------
Data-parallel over batch: each core takes 32 of the 256 sequences, and
runs the fwd chain (t=0..255) and the bwd chain (t=511..256, states
reversed) together as 64 rows of one transposed-layout DP.

The DP runs in PROBABILITY space: with (b,dir) on SBUF partitions and
the extended-label state s on the free dimension, one time step is

    S'[r, s] = q_j[r,s] * (S[r,s] + S[r,s-1]) + qg_j[r,s] * S[r,s-2]

where q = y_pred[., t, ext[s]] + eps (gathered emission probs) and
qg = q * skip-gate.  The state shifts are free-dim AP offsets (an
overlapping stride -1 view), so a step is 4 bf16 DVE instructions and
nothing else -- no matmuls, no PSUM, no log/exp in the serial chain.

fp32/bf16 range is handled by a flush every KF steps: the max of S
over a window around the wavefront diagonal (s ~ j/2) is rescaled to
2^BIAS by an exact power of two (exponent bit arithmetic on DVE), a
high cap protects runaway leader states, and the applied log2-scale
accumulates per row.  States that underflow relative to the window are
> e^-45 below every contributing path -- dropping them is harmless at
the 2e-2 tolerance (validated vs the f32 reference at ~6e-3 max rel
err over all 256 examples in numpy simulation of this exact bf16
arithmetic).

The q/qg tables are produced on device, overlapped with the DP: the
host supplies y transposed to [v, t] (and a time-reversed copy for the
bwd chains) in bf16; one-hot gather matmuls (eps folded in, the skip
gate folded into the qg columns) produce [t, 2, 128] tiles in PSUM, an
ACT/DVE copy moves them to SBUF bf16, and one SWDGE DMA per
(b,dir,chunk) scatters rows into the per-row [j, 2, 128] qcat layout.

Host combine in f64: loss = -logsumexp_s(alpha[s] + betahat[s]),
alpha = log(S_fwd) - E_fwd*log 2, exactly as the validated v1 combine.
"""

import sys

sys.path.insert(0, "/opt/trn_rl_repo")

from contextlib import ExitStack

import numpy as np
import ml_dtypes

import concourse.bass as bass
import concourse.tile as tile
from concourse import bacc, mybir
from concourse.ap import AP
from concourse.bass_utils import run_bass_kernel_spmd

bf16 = ml_dtypes.bfloat16

B, T, V, L = 256, 512, 256, 64
S = 2 * L + 1            # 129 extended states; chains keep 128 each
BLANK = V - 1
EPS = 1e-7
NCORES = 8
BPC = B // NCORES        # 32 examples per core
NJ = T // 2              # 256 time steps per chain
KF = 8                   # flush period
BIAS = 64                # flush rescales window max to 2^BIAS
WIN = 24                 # flush window half-width around the diagonal
CAPF = float(2.0 ** 101)
FP32 = mybir.dt.float32
BF16 = mybir.dt.bfloat16
I32 = mybir.dt.int32
ALU = mybir.AluOpType


def _kernel_body(ctx, tc, ytf_in, ytr_in, g_in, s_out, e_out):
    nc = tc.nc

    const_pool = ctx.enter_context(tc.tile_pool(name="const", bufs=1))
    g_pool = ctx.enter_context(tc.tile_pool(name="gmat", bufs=2))
    qcat_pool = ctx.enter_context(tc.tile_pool(name="qcat", bufs=1))
    yt_pool = ctx.enter_context(tc.tile_pool(name="yt", bufs=3))
    qs_pool = ctx.enter_context(tc.tile_pool(name="qs", bufs=10))
    psum_g = ctx.enter_context(tc.tile_pool(name="pg", bufs=8, space="PSUM"))
    work = ctx.enter_context(tc.tile_pool(name="work", bufs=4))

    # q tables: per row r=(d,b), per step j: [2, 128] (q | q*gate)
    qcat = qcat_pool.tile([64, NJ, 2, 128], BF16)

    # ---- production --------------------------------------------------
    # group ch covers j in [ch*128, (ch+1)*128): fwd rows consume ytf
    # t-chunk ch, bwd rows consume ytr t-chunk ch (already reversed).
    YB = 8                       # examples per yt slice DMA
    GB = 8                       # examples per gm slice DMA

    def produce_group(ch):
        _save = tc.cur_priority
        tc.cur_priority = _save + 1_000_000
        for b0 in range(0, BPC, GB):
            gm = g_pool.tile([128, GB, 2, 2, 256], BF16, tag="gm")
            ga = g_in[b0:b0 + GB]
            gsrc = AP(ga.tensor, ga.offset,
                      [[1024, 128], [128 * 1024, GB], [1, 1024]])
            nc.sync.dma_start(gm[:], gsrc)
            yts = {}
            for d_ in range(2):
                if b0 % YB == 0:
                    for h in range(2):
                        yt_ = yt_pool.tile([128, YB, 128], BF16,
                                           tag=f"yt{d_}{h}")
                        ya = (ytf_in if d_ == 0 else ytr_in)
                        off = (h * 128 * BPC * T + b0 * T + ch * 128)
                        src = AP(ya.tensor, off,
                                 [[BPC * T, 128], [T, YB], [1, 128]])
                        if ch == 0 and d_ == 0:
                            nc.scalar.dma_start(yt_[:], src)
                        else:
                            nc.sync.dma_start(yt_[:], src)
                        yts[(d_, h)] = yt_
                        produce_group.yts[(d_, h)] = yt_
                else:
                    yts = produce_group.yts
            for bi in range(GB):
                b_ = b0 + bi
                for d_ in range(2):
                    yth = produce_group.yts
                    gps = psum_g.tile([128, 256], FP32, tag="gps")
                    for h in range(2):
                        nc.tensor.matmul(gps[:],
                                         yth[(d_, h)][:, b_ % YB, :],
                                         gm[:, bi, d_, h, :],
                                         start=(h == 0), stop=(h == 1))
                    qs = qs_pool.tile([128, 2, 128], BF16, tag="qs")
                    if ch == 0 and (b_ + d_) % 2 == 0:
                        nc.vector.tensor_copy(qs[:], gps[:])
                    else:
                        nc.scalar.copy(qs[:], gps[:])
                    r_ = d_ * BPC + b_
                    a = qcat[r_:r_ + 1, ch * 128:(ch + 1) * 128, :, :]
                    dst = AP(a.tensor, a.offset,
                             [list(a.ap[0]), [256, 128], [1, 256]])
                    if ch == 0 and d_ == 1:
                        nc.sync.dma_start(dst, qs[:])
                    else:
                        nc.gpsimd.dma_start(dst, qs[:])
        tc.cur_priority = _save

    produce_group.yts = {}

    produce_group(0)

    # ---- DP chain ----------------------------------------------------
    SA = const_pool.tile([64, 130], BF16)
    SB = const_pool.tile([64, 130], BF16)
    eacc = const_pool.tile([64, 1], FP32)
    nc.vector.memset(SA[:], 0.0)
    nc.vector.memset(SB[:], 0.0)
    nc.vector.memset(eacc[:], 0.0)
    nc.vector.tensor_copy(SA[:, 2:4], qcat[:, 0, 0, 0:2])

    cur, nxt = SA, SB
    for j in range(1, NJ):
        if j == 64:
            produce_group(1)
        w = min(128, 2 * j + 2)   # wavefront: states s >= 2j+2 are still 0
        flush = (j % KF == 0 and j < NJ - 1)
        if flush:
            # scale factor from the PRE-step state (1 step stale; the
            # bias absorbs the bounded offset) so the bit chain hides
            # between the step's own ops instead of serializing after.
            s0 = j // 2
            lo, hi = max(0, s0 - WIN), min(128, s0 + WIN + 1)
            wm32 = work.tile([64, 1], FP32, tag="wm32")
            nc.vector.tensor_reduce(wm32[:], cur[:, 2 + lo:2 + hi],
                                    axis=mybir.AxisListType.X, op=ALU.max)
        qj = qcat[:, j]
        q0b = qj[:, 0, 0:w].unsqueeze(1).broadcast_to([64, 2, w])
        sap = cur[:]
        s2v = AP(sap.tensor, sap.offset + 2,
                 [list(sap.ap[0]), [-1, 2], [1, w]])
        m = work.tile([64, 3, 128], BF16, tag="m")
        nc.vector.tensor_mul(m[:, 0:2, 0:w], q0b, s2v)
        nc.vector.tensor_mul(m[:, 2, 0:w], qj[:, 1, 0:w], cur[:, 0:w])
        if flush:
            t1 = work.tile([64, 1], I32, tag="t1")
            nc.vector.tensor_scalar(t1[:], wm32[:].bitcast(I32), 23, -1,
                                    op0=ALU.logical_shift_right,
                                    op1=ALU.bitwise_xor)
        u = work.tile([64, 128], BF16, tag="u")
        nc.vector.tensor_add(u[:, 0:w], m[:, 0, 0:w], m[:, 1, 0:w])
        if flush:
            f = work.tile([64, 1], I32, tag="f")
            nc.vector.tensor_scalar(f[:], t1[:], BIAS + 255, 254,
                                    op0=ALU.add, op1=ALU.min)
        nc.vector.tensor_add(nxt[:, 2:2 + w], u[:, 0:w], m[:, 2, 0:w])
        if flush:
            nc.vector.scalar_tensor_tensor(eacc[:], f[:], -127.0, eacc[:],
                                           ALU.add, ALU.add)
            sc_i = work.tile([64, 1], I32, tag="sci")
            nc.vector.tensor_scalar(sc_i[:], f[:], 23, None,
                                    op0=ALU.logical_shift_left)
        cur, nxt = nxt, cur
        if flush:
            nc.vector.tensor_scalar(nxt[:], cur[:], sc_i[:].bitcast(FP32), CAPF,
                                    op0=ALU.mult, op1=ALU.min)
            cur, nxt = nxt, cur

    nc.sync.dma_start(s_out, cur[:])
    nc.sync.dma_start(e_out, eacc[:])


_CACHED = None


def _build():
    global _CACHED
    if _CACHED is not None:
        return _CACHED
    nc = bacc.Bacc("TRN2", target_bir_lowering=False, debug=False,
                   num_devices=NCORES)
    ytf_in = nc.dram_tensor("ytf", [2, 128, BPC, T], BF16,
                            kind="ExternalInput").ap()
    ytr_in = nc.dram_tensor("ytr", [2, 128, BPC, T], BF16,
                            kind="ExternalInput").ap()
    g_in = nc.dram_tensor("g", [BPC, 128, 2, 2, 256], BF16,
                          kind="ExternalInput").ap()
    s_out = nc.dram_tensor("souts", [64, 130], BF16, kind="ExternalOutput").ap()
    e_out = nc.dram_tensor("eouts", [64, 1], FP32, kind="ExternalOutput").ap()

    with tile.TileContext(nc) as tc:
        with ExitStack() as ctx:
            _kernel_body(ctx, tc, ytf_in, ytr_in, g_in, s_out, e_out)
    nc.compile()
    _CACHED = nc
    return nc


def _host_tensors(y_true, y_pred):
    """Per-core input dicts. Host does layout only: y transposed to
    [v,t] bf16 (plus a time-reversed copy) and one-hot gather matrices."""
    y_true = np.asarray(y_true)
    y_pred = np.asarray(y_pred, dtype=np.float32)

    in_maps = []
    for core in range(NCORES):
        bs = slice(core * BPC, (core + 1) * BPC)
        yt_c = y_true[bs]
        # [b, t, v] -> [h, v128, b, t] transposed bf16
        ytb = np.ascontiguousarray(
            y_pred[bs].transpose(2, 0, 1).reshape(2, 128, BPC, T)).astype(bf16)
        ytr = np.ascontiguousarray(ytb[:, :, :, ::-1])
        g = np.zeros((BPC, 128, 2, 2, 256), np.float32)
        for b_ in range(BPC):
            ext = np.full(S, BLANK, dtype=np.int64)
            ext[1::2] = yt_c[b_]
            extm2 = np.concatenate([np.full(2, -1, dtype=np.int64), ext[:-2]])
            skip = ((ext != BLANK) & (ext != extm2)).astype(np.float32)
            # fwd (d=0): col s = 0..127 from ext[s]
            gf = np.zeros(128, np.float32)
            gf[2:] = skip[2:128]
            vf = ext[0:128]
            # bwd (d=1): col r = 0..127 from ext[128-r]
            gb = np.zeros(128, np.float32)
            rarr = np.arange(2, 128)
            gb[rarr] = skip[130 - rarr]
            vb = ext[128 - np.arange(128)]
            for d_, vv, gg in ((0, vf, gf), (1, vb, gb)):
                for s_ in range(128):
                    v = int(vv[s_])
                    # q column: onehot + eps on every v row
                    g[b_, v % 128, d_, v // 128, s_] += 1.0
                    g[b_, :, d_, :, s_] += EPS
                    # qg column: (onehot + eps) * gate
                    if gg[s_] > 0:
                        g[b_, v % 128, d_, v // 128, 128 + s_] += 1.0
                        g[b_, :, d_, :, 128 + s_] += EPS
        in_maps.append({
            "ytf": ytb,
            "ytr": ytr,
            "g": g.astype(bf16),
        })
    return in_maps


def _combine(souts, eouts):
    """Host f64 combine: loss[b] = -logsumexp_s(alpha[s] + betahat[s])."""
    ln2 = np.log(2.0)
    loss = np.zeros(B, dtype=np.float64)
    with np.errstate(divide="ignore"):
        for core in range(NCORES):
            sv = souts[core].astype(np.float64)
            ev = eouts[core].astype(np.float64)
            for b_ in range(BPC):
                af = np.log(sv[b_, 2:130]) - ev[b_, 0] * ln2
                ab = np.log(sv[BPC + b_, 2:130]) - ev[BPC + b_, 0] * ln2
                ls = af[1:128] + ab[127:0:-1]
                fin = np.isfinite(ls)
                mm = ls[fin].max()
                loss[core * BPC + b_] = -(np.log(np.exp(ls[fin] - mm).sum()) + mm)
    return loss


def kernel(y_true, y_pred):
    nc = _build()
    in_maps = _host_tensors(y_true, y_pred)
    res = run_bass_kernel_spmd(nc, in_maps, list(range(NCORES)))
    souts = [np.asarray(res.results[i]["souts"]) for i in range(NCORES)]
    eouts = [np.asarray(res.results[i]["eouts"]) for i in range(NCORES)]
    loss = _combine(souts, eouts)
    return loss.astype(np.float32)[:, None]
